# revision 1
# baseline (speedup 1.0000x reference)
"""Causal self-attention (B=4, T=2048, C=1024, NH=16) on 8 TRN2 NeuronCores.

Sharding: core = 2*b + g  (b in 0..3 batches, g in 0..1 head-groups of 8 heads).
Each core computes qkv projection for its 8 heads, causal flash attention,
and a partial output projection (rows g*512:(g+1)*512 of w_proj).  Host sums
the two partials per batch and adds b_proj.

Layouts on device (per core):
  qT, kT : [head-dims on partitions, T on free]  (from  W.T @ x.T  matmuls)
  v      : natural [T on partitions, head-dims on free], with a ones-column
           appended per head so the PV matmul also produces the softmax
           denominator (lhsT = [v_h | 1] -> out rows 0..63 = y^T, row 64 = sum)
  S^T    : [keys on partitions, queries on free]; exp on ScalarE (no max
           subtraction needed: |S/8| <~ 6 for N(0,1) logits), causal mask via
           gpsimd affine_select on the 4 diagonal tiles per query block.
"""

import numpy as np

import concourse.bass as bass
import concourse.mybir as mybir
import concourse.tile as tile
from concourse import bacc
from concourse.bass_utils import run_bass_kernel_spmd
from concourse.masks import make_identity

B, T, C = 4, 2048, 1024
NH, HD = 16, 64
G = 2              # head groups (cores per batch)
HPG = NH // G      # heads per group = 8
GD = HPG * HD      # dims per group = 512
N_CORES = B * G

FP32 = mybir.dt.float32

# matmul dtype mode: "f32" (exact, 4 cyc/row), "f32r" (1 cyc/row at N>=256),
# "bf16" (1 cyc/row, operands stored bf16)
MM_MODE = "bf16"


def _st_dt():
    """dtype of SBUF tiles that feed TensorE matmuls (walrus requires
    float32r-producing instructions for fp32r matmul operands)."""
    if MM_MODE == "bf16":
        return mybir.dt.bfloat16
    if MM_MODE == "f32r":
        return mybir.dt.float32r
    return FP32


def _xn_dt():
    """dtype of the x-natural tiles / PE-transpose path (plain f32 there)."""
    return mybir.dt.bfloat16 if MM_MODE == "bf16" else FP32


def _mm(ap):
    return ap


def build_nc():
    ST = _st_dt()
    XN = _xn_dt()
    nc = bacc.Bacc()

    x = nc.declare_dram_parameter("x", [T, C], XN, isOutput=False)
    wq = nc.declare_dram_parameter("wq", [C, GD], ST, isOutput=False)
    wk = nc.declare_dram_parameter("wk", [C, GD], ST, isOutput=False)
    wv = nc.declare_dram_parameter("wv", [C, GD], ST, isOutput=False)
    bq = nc.declare_dram_parameter("bq", [GD], FP32, isOutput=False)
    bk = nc.declare_dram_parameter("bk", [GD], FP32, isOutput=False)
    bv = nc.declare_dram_parameter("bv", [GD], FP32, isOutput=False)
    wp = nc.declare_dram_parameter("wp", [GD, C], ST, isOutput=False)
    ones = nc.declare_dram_parameter("ones", [128, HPG], ST, isOutput=False)
    out = nc.declare_dram_parameter("out", [T, C], FP32, isOutput=True)

    NCC = C // 128      # 8 contraction chunks for the qkv projection
    NMB = GD // 128     # 4 blocks of 128 qkv-dims per section
    NTB = T // 512      # 4 T-blocks of 512
    NKC = T // 128      # 16 key chunks of 128

    from contextlib import ExitStack

    with tile.TileContext(nc) as tc, ExitStack() as stack:
        consts = stack.enter_context(tc.tile_pool(name="consts", bufs=1))
        persist = stack.enter_context(tc.tile_pool(name="persist", bufs=1))

        if MM_MODE != "bf16":
            ident = consts.tile([128, 128], XN, tag="ident")
            make_identity(nc, ident)
        bq_col = consts.tile([128, NMB], FP32, tag="bq_col")
        bk_col = consts.tile([128, NMB], FP32, tag="bk_col")
        for m in range(NMB):
            nc.sync.dma_start(out=bq_col[:, m : m + 1], in_=bq[bass.ts(m, 128)])
            nc.sync.dma_start(out=bk_col[:, m : m + 1], in_=bk[bass.ts(m, 128)])
        # bv broadcast to all 128 partitions (DMA supports partition step 0)
        bv_bc = consts.tile([128, GD], FP32, tag="bv_bc")
        nc.sync.dma_start(out=bv_bc, in_=bv[None, :].partition_broadcast(128))

        # persistent activations
        qT_t = [persist.tile([128, T], ST, tag=f"qT{m}", name=f"qT{m}") for m in range(NMB)]
        kT_t = [persist.tile([128, T], ST, tag=f"kT{m}", name=f"kT{m}") for m in range(NMB)]
        v_all = persist.tile([128, NKC, HPG, HD + 1], ST, tag="v_all", name="v_all")
        # y^T reuses the qT tiles: the query columns of head-pair m, block qb
        # are dead once that block's PV matmuls have consumed them.
        yT_t = qT_t

        # ---------------- Stage A: qkv projection ----------------
        with (
            tc.tile_pool(name="wA", bufs=1) as wA_pool,
            tc.tile_pool(name="xA", bufs=3) as xA_pool,
            tc.tile_pool(name="xT", bufs=8) as xT_pool,
            tc.tile_pool(name="trps", bufs=2, space="PSUM") as trps_pool,
            tc.tile_pool(name="qkvps", bufs=4, space="PSUM") as qkvps_pool,
        ):
            wq_t = wA_pool.tile([128, NCC, GD], ST, tag="wq")
            wk_t = wA_pool.tile([128, NCC, GD], ST, tag="wk")
            wv_t = wA_pool.tile([128, NCC, GD], ST, tag="wv")
            for c in range(NCC):
                nc.sync.dma_start(out=wq_t[:, c, :], in_=wq[bass.ts(c, 128), :])
                nc.sync.dma_start(out=wk_t[:, c, :], in_=wk[bass.ts(c, 128), :])
                nc.sync.dma_start(out=wv_t[:, c, :], in_=wv[bass.ts(c, 128), :])

            for tb in range(NTB):
                # x^T chunks for this block of 512 timesteps
                xtc = [xT_pool.tile([128, 512], ST, tag="xtc", name="xtc") for _ in range(NCC)]
                if MM_MODE == "bf16":
                    # 2-byte dtype: hardware xbar DMA transpose straight from DRAM
                    for c in range(NCC):
                        nc.sync.dma_start(
                            out=xtc[c],
                            in_=x[bass.ts(tb, 512), bass.ts(c, 128)],
                            transpose=True,
                        )
                else:
                    for tsub in range(4):
                        xn = xA_pool.tile([128, C], XN, tag="xn")
                        t0 = tb * 512 + tsub * 128
                        nc.sync.dma_start(out=xn, in_=x[t0 : t0 + 128, :])
                        for c in range(NCC):
                            trp = trps_pool.tile([128, 128], XN, tag="trp")
                            nc.tensor.transpose(trp, xn[:, bass.ts(c, 128)], ident)
                            nc.vector.tensor_copy(
                                xtc[c][:, bass.ts(tsub, 128)], trp
                            )

                # q^T and k^T for this T-block
                for w_t, b_col, dst in ((wq_t, bq_col, qT_t), (wk_t, bk_col, kT_t)):
                    for m in range(NMB):
                        ps = qkvps_pool.tile([128, 512], FP32, tag="qkvps")
                        for c in range(NCC):
                            nc.tensor.matmul(
                                ps,
                                _mm(w_t[:, c, bass.ts(m, 128)]),
                                _mm(xtc[c]),
                                start=(c == 0),
                                stop=(c == NCC - 1),
                            )
                        nc.scalar.activation(
                            out=dst[m][:, bass.ts(tb, 512)],
                            in_=ps,
                            func=mybir.ActivationFunctionType.Identity,
                            bias=b_col[:, m : m + 1],
                        )

                # v natural for this T-block (4 key chunks of 128)
                for tsub in range(4):
                    kc = tb * 4 + tsub
                    ps = qkvps_pool.tile([128, GD], FP32, tag="qkvps")
                    for c in range(NCC):
                        nc.tensor.matmul(
                            ps,
                            _mm(xtc[c][:, bass.ts(tsub, 128)]),
                            _mm(wv_t[:, c, :]),
                            start=(c == 0),
                            stop=(c == NCC - 1),
                        )
                    vt = v_all[:, kc, :, :]
                    nc.vector.tensor_add(
                        vt[:, :, 0:HD],
                        ps.rearrange("p (h d) -> p h d", h=HPG),
                        bv_bc.rearrange("p (h d) -> p h d", h=HPG),
                    )
                    nc.sync.dma_start(
                        out=vt[:, :, HD : HD + 1], in_=ones[:, :, None]
                    )

        # ---------------- Stage B: causal attention + interleaved proj ----------------
        PDT = _st_dt()
        with (
            tc.tile_pool(name="pT", bufs=10) as pT_pool,
            tc.tile_pool(name="rec", bufs=4) as rec_pool,
            tc.tile_pool(name="wp", bufs=1) as wp_pool,
            tc.tile_pool(name="osb", bufs=4) as osb_pool,
            tc.tile_pool(name="sps", bufs=3, space="PSUM") as sps_pool,
            tc.tile_pool(name="pvps", bufs=3, space="PSUM") as pvps_pool,
            tc.tile_pool(name="ops", bufs=2, space="PSUM") as ops_pool,
        ):
            wp_t = wp_pool.tile([128, NMB, C], ST, tag="wp")
            for c in range(NMB):
                nc.sync.dma_start(out=wp_t[:, c, :], in_=wp[bass.ts(c, 128), :])

            dens = {}
            for qb in range(NTB):
                kcmax = (qb + 1) * 4
                for m in range(NMB):
                    if qb == 0:
                        dens[m] = rec_pool.tile(
                            [64, 512], FP32, tag=f"den{m}", name=f"den{m}", bufs=1
                        )
                        nc.vector.memset(dens[m], 1.0)
                    den = dens[m]
                    pvs = [
                        pvps_pool.tile([HD + 1, 512], FP32, tag="pvps", name="pvps")
                        for _ in range(2)
                    ]
                    for kc in range(kcmax):
                        pTs = []
                        for hp in range(2):
                            base = hp * 64
                            sp = sps_pool.tile([128, 512], FP32, tag="sps")
                            nc.tensor.matmul(
                                sp,
                                _mm(kT_t[m][base : base + 64, bass.ts(kc, 128)]),
                                _mm(qT_t[m][base : base + 64, bass.ts(qb, 512)]),
                                start=True,
                                stop=True,
                            )
                            pT = pT_pool.tile([128, 512], PDT, tag="pT")
                            nc.scalar.activation(
                                out=pT,
                                in_=sp,
                                func=mybir.ActivationFunctionType.Exp,
                                scale=1.0 / float(np.sqrt(HD)),
                            )
                            r = kc - qb * 4
                            if r >= 0:
                                # keep key j <= query i:  (il - jl - 128 r) >= 0
                                nc.gpsimd.affine_select(
                                    out=pT,
                                    in_=pT,
                                    compare_op=mybir.AluOpType.is_ge,
                                    fill=0.0,
                                    base=-128 * r,
                                    channel_multiplier=-1,
                                    pattern=[[1, 512]],
                                )
                            pTs.append(pT)
                        for hp in range(2):
                            h = 2 * m + hp
                            nc.tensor.matmul(
                                pvs[hp],
                                _mm(v_all[:, kc, h, :]),
                                _mm(pTs[hp]),
                                start=(kc == 0),
                                stop=(kc == kcmax - 1),
                            )
                    for hp in range(2):
                        base = hp * 64
                        # unnormalized y^T and denominator row; normalize below
                        nc.vector.tensor_copy(
                            yT_t[m][base : base + 64, bass.ts(qb, 512)],
                            pvs[hp][0:HD, :],
                        )
                        nc.vector.tensor_copy(
                            den[32 * hp : 32 * hp + 1, :],
                            pvs[hp][HD : HD + 1, :],
                        )
                # normalize all head-pairs for this qb (batched reciprocal per m
                # amortizes the DVE per-free-element reciprocal cost)
                for m in range(NMB):
                    den = dens[m]
                    denr = rec_pool.tile([64, 512], FP32, tag=f"denr{m}", name=f"denr{m}", bufs=2)
                    nc.vector.reciprocal(denr, den)
                    for hp in range(2):
                        base = hp * 64
                        if hp == 0:
                            src_row = denr[0:1, :]
                        else:
                            dtmp = rec_pool.tile([1, 512], FP32, tag="dtmp", name="dtmp")
                            nc.vector.tensor_copy(dtmp, denr[32:33, :])
                            src_row = dtmp
                        rbc = rec_pool.tile([128, 512], FP32, tag="rbc", name="rbc")
                        nc.gpsimd.partition_broadcast(rbc, src_row)
                        nc.vector.tensor_mul(
                            yT_t[m][base : base + 64, bass.ts(qb, 512)],
                            yT_t[m][base : base + 64, bass.ts(qb, 512)],
                            rbc[base : base + 64, :],
                        )
                # output projection for the 4 T-subblocks of this query block
                for tsub in range(4):
                    tb16 = qb * 4 + tsub
                    for nb in range(C // 512):
                        ps = ops_pool.tile([128, 512], FP32, tag="ops")
                        for c in range(NMB):
                            nc.tensor.matmul(
                                ps,
                                _mm(yT_t[c][:, bass.ts(tb16, 128)]),
                                _mm(wp_t[:, c, bass.ts(nb, 512)]),
                                start=(c == 0),
                                stop=(c == NMB - 1),
                            )
                        osb = osb_pool.tile([128, 512], FP32, tag="osb")
                        nc.scalar.copy(osb, ps)
                        nc.sync.dma_start(
                            out=out[bass.ts(tb16, 128), bass.ts(nb, 512)], in_=osb
                        )

    nc.compile()
    return nc


_CACHE = {}


def _get_nc():
    if "nc" not in _CACHE:
        _CACHE["nc"] = build_nc()
    return _CACHE["nc"]


def _to_st(a):
    a = np.asarray(a, dtype=np.float32)
    if MM_MODE == "bf16":
        import ml_dtypes

        return np.ascontiguousarray(a.astype(ml_dtypes.bfloat16))
    return np.ascontiguousarray(a)


def make_in_maps(x, w_qkv, b_qkv, w_proj):
    x = np.asarray(x, dtype=np.float32)
    w_qkv = np.asarray(w_qkv, dtype=np.float32)
    b_qkv = np.asarray(b_qkv, dtype=np.float32)
    in_maps = []
    for core in range(N_CORES):
        b, g = divmod(core, G)
        in_maps.append(
            {
                "x": _to_st(x[b]),
                "wq": _to_st(w_qkv[:, GD * g : GD * g + GD]),
                "wk": _to_st(w_qkv[:, C + GD * g : C + GD * g + GD]),
                "wv": _to_st(w_qkv[:, 2 * C + GD * g : 2 * C + GD * g + GD]),
                "bq": np.ascontiguousarray(b_qkv[GD * g : GD * g + GD]),
                "bk": np.ascontiguousarray(b_qkv[C + GD * g : C + GD * g + GD]),
                "bv": np.ascontiguousarray(b_qkv[2 * C + GD * g : 2 * C + GD * g + GD]),
                "wp": _to_st(np.asarray(w_proj, dtype=np.float32)[GD * g : GD * g + GD, :]),
                "ones": _to_st(np.ones((128, HPG), dtype=np.float32)),
            }
        )
    return in_maps


def _assemble(results, b_proj):
    y = np.empty((B, T, C), dtype=np.float32)
    for b in range(B):
        y[b] = results[G * b]["out"] + results[G * b + 1]["out"]
    y += np.asarray(b_proj, dtype=np.float32)[None, None, :]
    return y


def kernel(x, w_qkv, b_qkv, w_proj, b_proj):
    nc = _get_nc()
    in_maps = make_in_maps(x, w_qkv, b_qkv, w_proj)
    res = run_bass_kernel_spmd(nc, in_maps, list(range(N_CORES)))
    return _assemble(res.results, b_proj)



# revision 4
# speedup vs baseline: 1.3307x; 1.3307x over previous
"""Causal self-attention (B=4, T=2048, C=1024, NH=16) on 8 TRN2 NeuronCores.

Sharding: core = 2*b + g  (b in 0..3 batches, g in 0..1 head-groups of 8 heads).
Each core computes qkv projection for its 8 heads, causal flash attention,
and a partial output projection (rows g*512:(g+1)*512 of w_proj).  Host sums
the two partials per batch and adds b_proj.

Layouts on device (per core):
  qT, kT : [head-dims on partitions, T on free]  (from  W.T @ x.T  matmuls)
  v      : natural [T on partitions, head-dims on free], with a ones-column
           appended per head so the PV matmul also produces the softmax
           denominator (lhsT = [v_h | 1] -> out rows 0..63 = y^T, row 64 = sum)
  S^T    : [keys on partitions, queries on free]; exp on ScalarE (no max
           subtraction needed: |S/8| <~ 6 for N(0,1) logits), causal mask via
           gpsimd affine_select on the diagonal tiles per query block.

Schedule (v2): single fused region.  Attention for query-block qb is
software-pipelined (S^T of step i+1 issued before PV of step i, so the PE
never waits on the ScalarE exp), with the qkv projection of t-block qb+1 and
the output projection of query-block qb-1 injected between attention steps as
filler matmuls to keep the PE saturated.  Exp is batched over both heads of a
head-pair ([128,1024] PSUM tile -> one ACTIVATE).  All PSUM evictions run on
DVE/ScalarE off the PE critical path.
"""

import numpy as np

import concourse.bass as bass
import concourse.mybir as mybir
import concourse.tile as tile
from concourse import bacc
from concourse.bass_utils import run_bass_kernel_spmd

B, T, C = 4, 2048, 1024
NH, HD = 16, 64
G = 2              # head groups (cores per batch)
HPG = NH // G      # heads per group = 8
GD = HPG * HD      # dims per group = 512
N_CORES = B * G

FP32 = mybir.dt.float32
BF16 = mybir.dt.bfloat16

NCC = C // 128      # 8 contraction chunks for the qkv projection
NMB = GD // 128     # 4 blocks of 128 qkv-dims per section (head-pairs)
NTB = T // 512      # 4 T-blocks of 512
NKC = T // 128      # 16 key chunks of 128


def build_nc():
    nc = bacc.Bacc()

    x = nc.declare_dram_parameter("x", [T, C], BF16, isOutput=False)
    wq = nc.declare_dram_parameter("wq", [C, GD], BF16, isOutput=False)
    wk = nc.declare_dram_parameter("wk", [C, GD], BF16, isOutput=False)
    wv = nc.declare_dram_parameter("wv", [C, GD], BF16, isOutput=False)
    bq = nc.declare_dram_parameter("bq", [GD], FP32, isOutput=False)
    bk = nc.declare_dram_parameter("bk", [GD], FP32, isOutput=False)
    bv = nc.declare_dram_parameter("bv", [GD], FP32, isOutput=False)
    wp = nc.declare_dram_parameter("wp", [GD, C], BF16, isOutput=False)
    ones = nc.declare_dram_parameter("ones", [128, HPG], BF16, isOutput=False)
    out = nc.declare_dram_parameter("out", [T, C], FP32, isOutput=True)

    from contextlib import ExitStack

    with tile.TileContext(nc) as tc, ExitStack() as stack:
        consts = stack.enter_context(tc.tile_pool(name="consts", bufs=1))
        persist = stack.enter_context(tc.tile_pool(name="persist", bufs=1))
        wA_pool = stack.enter_context(tc.tile_pool(name="wA", bufs=1))
        xT_pool = stack.enter_context(tc.tile_pool(name="xT", bufs=16))
        pT_pool = stack.enter_context(tc.tile_pool(name="pT", bufs=4))
        nrm_pool = stack.enter_context(tc.tile_pool(name="nrm", bufs=4))
        osb_pool = stack.enter_context(tc.tile_pool(name="osb", bufs=4))
        accps_pool = stack.enter_context(
            tc.tile_pool(name="accps", bufs=2, space="PSUM")
        )
        sps_pool = stack.enter_context(tc.tile_pool(name="sps", bufs=2, space="PSUM"))
        pvps_pool = stack.enter_context(
            tc.tile_pool(name="pvps", bufs=2, space="PSUM")
        )

        # ---- constants / biases (scalar queue: parallel with x transposes) ----
        bq_col = consts.tile([128, NMB], FP32, tag="bq_col")
        bk_col = consts.tile([128, NMB], FP32, tag="bk_col")
        for m in range(NMB):
            nc.scalar.dma_start(out=bq_col[:, m : m + 1], in_=bq[bass.ts(m, 128)])
            nc.scalar.dma_start(out=bk_col[:, m : m + 1], in_=bk[bass.ts(m, 128)])
        bv_bc = consts.tile([128, GD], FP32, tag="bv_bc")
        nc.scalar.dma_start(out=bv_bc, in_=bv[None, :].partition_broadcast(128))

        # ---- weights (scalar queue) ----
        wq_t = wA_pool.tile([128, NCC, GD], BF16, tag="wq")
        wk_t = wA_pool.tile([128, NCC, GD], BF16, tag="wk")
        wv_t = wA_pool.tile([128, NCC, GD], BF16, tag="wv")
        wp_t = wA_pool.tile([128, NMB, C], BF16, tag="wp")
        for c in range(NCC):
            nc.scalar.dma_start(out=wq_t[:, c, :], in_=wq[bass.ts(c, 128), :])
            nc.scalar.dma_start(out=wk_t[:, c, :], in_=wk[bass.ts(c, 128), :])
            nc.scalar.dma_start(out=wv_t[:, c, :], in_=wv[bass.ts(c, 128), :])
        for c in range(NMB):
            nc.scalar.dma_start(out=wp_t[:, c, :], in_=wp[bass.ts(c, 128), :])

        # ---- persistent activations ----
        qT_t = [persist.tile([128, T], BF16, tag=f"qT{m}", name=f"qT{m}") for m in range(NMB)]
        kT_t = [persist.tile([128, T], BF16, tag=f"kT{m}", name=f"kT{m}") for m in range(NMB)]
        yT_t = [persist.tile([128, T], BF16, tag=f"yT{m}", name=f"yT{m}") for m in range(NMB)]
        v_all = persist.tile([128, NKC, HPG, HD + 1], BF16, tag="v_all")

        # ---------------- emission helpers ----------------

        def xtc_dma(tb):
            """Issue the x^T transpose DMAs for t-block tb; returns tiles."""
            xtc = [
                xT_pool.tile([128, 512], BF16, tag="xtc", name=f"xtc{tb}_{c}")
                for c in range(NCC)
            ]
            for c in range(NCC):
                nc.sync.dma_start(
                    out=xtc[c],
                    in_=x[bass.ts(tb, 512), bass.ts(c, 128)],
                    transpose=True,
                )
            return xtc

        def qkv_ops(tb, xtc):
            """One-PE-matmul callables for the qkv projection of t-block tb."""
            ops = []

            def qk_chain(w_t, b_col, dst, m):
                ps = accps_pool.tile([128, 512], FP32, tag="accps", name="accps")

                def mk(c):
                    def op():
                        nc.tensor.matmul(
                            ps,
                            w_t[:, c, bass.ts(m, 128)],
                            xtc[c],
                            start=(c == 0),
                            stop=(c == NCC - 1),
                        )
                        if c == NCC - 1:
                            nc.vector.tensor_scalar_add(
                                dst[m][:, bass.ts(tb, 512)], ps, b_col[:, m : m + 1]
                            )

                    return op

                return [mk(c) for c in range(NCC)]

            def v_chain(tsub):
                kc = tb * 4 + tsub
                ps = accps_pool.tile([128, GD], FP32, tag="accps", name="accps")

                def mk(c):
                    def op():
                        nc.tensor.matmul(
                            ps,
                            xtc[c][:, bass.ts(tsub, 128)],
                            wv_t[:, c, :],
                            start=(c == 0),
                            stop=(c == NCC - 1),
                        )
                        if c == NCC - 1:
                            vt = v_all[:, kc, :, :]
                            nc.vector.tensor_add(
                                vt[:, :, 0:HD],
                                ps.rearrange("p (h d) -> p h d", h=HPG),
                                bv_bc.rearrange("p (h d) -> p h d", h=HPG),
                            )
                            nc.sync.dma_start(
                                out=vt[:, :, HD : HD + 1], in_=ones[:, :, None]
                            )

                    return op

                return [mk(c) for c in range(NCC)]

            for m in range(NMB):
                ops += qk_chain(wk_t, bk_col, kT_t, m)
                ops += qk_chain(wq_t, bq_col, qT_t, m)
                ops += v_chain(m)
            return ops

        def proj_ops(qb):
            """One-PE-matmul callables for the output projection of block qb."""
            ops = []
            for tsub in range(4):
                tb16 = qb * 4 + tsub
                for nb in range(C // 512):
                    ps = accps_pool.tile([128, 512], FP32, tag="accps", name="accps")

                    def mk(ps, tb16, nb, c):
                        def op():
                            nc.tensor.matmul(
                                ps,
                                yT_t[c][:, bass.ts(tb16, 128)],
                                wp_t[:, c, bass.ts(nb, 512)],
                                start=(c == 0),
                                stop=(c == NMB - 1),
                            )
                            if c == NMB - 1:
                                osb = osb_pool.tile([128, 512], FP32, tag="osb", name="osb")
                                nc.scalar.copy(osb, ps)
                                nc.sync.dma_start(
                                    out=out[bass.ts(tb16, 128), bass.ts(nb, 512)],
                                    in_=osb,
                                )

                        return op

                    ops += [mk(ps, tb16, nb, c) for c in range(NMB)]
            return ops

        # ---------------- attention ----------------
        scale = 1.0 / float(np.sqrt(HD))

        def attention(qb, fillers):
            kcmax = (qb + 1) * 4
            steps = [(m, kc) for m in range(NMB) for kc in range(kcmax)]
            nsteps = len(steps)
            pvs_by_m = {}
            pT_by = {}

            def emit_S(m, kc):
                sp = sps_pool.tile([128, 1024], FP32, tag="sps", name="sps")
                for hp in range(2):
                    base = hp * 64
                    nc.tensor.matmul(
                        sp[:, bass.ts(hp, 512)],
                        kT_t[m][base : base + 64, bass.ts(kc, 128)],
                        qT_t[m][base : base + 64, bass.ts(qb, 512)],
                        start=True,
                        stop=True,
                    )
                pT = pT_pool.tile([128, 1024], BF16, tag="pT", name="pT")
                nc.scalar.activation(
                    out=pT, in_=sp, func=mybir.ActivationFunctionType.Exp, scale=scale
                )
                r = kc - qb * 4
                if r >= 0:
                    # keep key j <= query i within the diagonal stripe; columns
                    # beyond 128*(r+1) are already fully valid.
                    w = 128 * (r + 1)
                    for hp in range(2):
                        sl = pT[:, 512 * hp : 512 * hp + w]
                        nc.gpsimd.affine_select(
                            out=sl,
                            in_=sl,
                            compare_op=mybir.AluOpType.is_ge,
                            fill=0.0,
                            base=-128 * r,
                            channel_multiplier=-1,
                            pattern=[[1, w]],
                        )
                pT_by[(m, kc)] = pT

            def emit_P(m, kc):
                if kc == 0:
                    pvs_by_m[m] = [
                        pvps_pool.tile([HD + 1, 512], FP32, tag="pvps", name="pvps")
                        for _ in range(2)
                    ]
                pvs = pvs_by_m[m]
                pT = pT_by.pop((m, kc))
                for hp in range(2):
                    nc.tensor.matmul(
                        pvs[hp],
                        v_all[:, kc, 2 * m + hp, :],
                        pT[:, bass.ts(hp, 512)],
                        start=(kc == 0),
                        stop=(kc == kcmax - 1),
                    )
                if kc == kcmax - 1:
                    # evict y^T (unnormalized) and normalize by the softmax
                    # denominator accumulated in row 64.
                    for hp in range(2):
                        base = hp * 64
                        ycols = yT_t[m][base : base + 64, bass.ts(qb, 512)]
                        nc.vector.tensor_copy(ycols, pvs[hp][0:HD, :])
                        den_s = nrm_pool.tile([1, 512], FP32, tag="den_s", name="den_s")
                        nc.vector.tensor_copy(den_s, pvs[hp][HD : HD + 1, :])
                        denr = nrm_pool.tile([1, 512], FP32, tag="denr", name="denr")
                        nc.vector.reciprocal_approx_fast(out=denr, in_=den_s)
                        rbc = nrm_pool.tile([128, 512], FP32, tag="rbc", name="rbc")
                        nc.gpsimd.partition_broadcast(rbc, denr)
                        nc.vector.tensor_mul(ycols, ycols, rbc[base : base + 64, :])

            nfill = len(fillers)
            fi = 0
            prev = None
            for i, st in enumerate(steps):
                emit_S(*st)
                # inject filler PE work between the S^T and the dependent PV
                want = (nfill * (i + 1)) // nsteps
                while fi < want:
                    fillers[fi]()
                    fi += 1
                if prev is not None:
                    emit_P(*prev)
                prev = st
            while fi < nfill:
                fillers[fi]()
                fi += 1
            emit_P(*prev)

        # ---------------- top-level schedule ----------------
        xtc_cur = xtc_dma(0)
        for op in qkv_ops(0, xtc_cur):
            op()
        for qb in range(NTB):
            fillers = []
            if qb < NTB - 1:
                xtc_nxt = xtc_dma(qb + 1)
                fillers += qkv_ops(qb + 1, xtc_nxt)
            if qb > 0:
                fillers += proj_ops(qb - 1)
            attention(qb, fillers)
        for op in proj_ops(NTB - 1):
            op()

    nc.compile()
    return nc


_CACHE = {}


def _get_nc():
    if "nc" not in _CACHE:
        _CACHE["nc"] = build_nc()
    return _CACHE["nc"]


def _to_bf16(a):
    import ml_dtypes

    a = np.asarray(a, dtype=np.float32)
    return np.ascontiguousarray(a.astype(ml_dtypes.bfloat16))


def make_in_maps(x, w_qkv, b_qkv, w_proj):
    x = np.asarray(x, dtype=np.float32)
    w_qkv = np.asarray(w_qkv, dtype=np.float32)
    b_qkv = np.asarray(b_qkv, dtype=np.float32)
    in_maps = []
    for core in range(N_CORES):
        b, g = divmod(core, G)
        in_maps.append(
            {
                "x": _to_bf16(x[b]),
                "wq": _to_bf16(w_qkv[:, GD * g : GD * g + GD]),
                "wk": _to_bf16(w_qkv[:, C + GD * g : C + GD * g + GD]),
                "wv": _to_bf16(w_qkv[:, 2 * C + GD * g : 2 * C + GD * g + GD]),
                "bq": np.ascontiguousarray(b_qkv[GD * g : GD * g + GD]),
                "bk": np.ascontiguousarray(b_qkv[C + GD * g : C + GD * g + GD]),
                "bv": np.ascontiguousarray(b_qkv[2 * C + GD * g : 2 * C + GD * g + GD]),
                "wp": _to_bf16(np.asarray(w_proj, dtype=np.float32)[GD * g : GD * g + GD, :]),
                "ones": _to_bf16(np.ones((128, HPG), dtype=np.float32)),
            }
        )
    return in_maps


def _assemble(results, b_proj):
    y = np.empty((B, T, C), dtype=np.float32)
    for b in range(B):
        y[b] = results[G * b]["out"] + results[G * b + 1]["out"]
    y += np.asarray(b_proj, dtype=np.float32)[None, None, :]
    return y


def kernel(x, w_qkv, b_qkv, w_proj, b_proj):
    nc = _get_nc()
    in_maps = make_in_maps(x, w_qkv, b_qkv, w_proj)
    res = run_bass_kernel_spmd(nc, in_maps, list(range(N_CORES)))
    return _assemble(res.results, b_proj)


# revision 10
# speedup vs baseline: 1.5050x; 1.1310x over previous
"""Causal self-attention (B=4, T=2048, C=1024, NH=16) on 8 TRN2 NeuronCores.

Sharding: core = 2*b + g  (b in 0..3 batches, g in 0..1 head-groups of 8 heads).
Each core computes qkv projection for its 8 heads, causal flash attention,
and a partial output projection (rows g*512:(g+1)*512 of w_proj).  Host sums
the two partials per batch and adds b_proj.

Layouts on device (per core):
  qT, kT : [head-dims on partitions, T on free]  (from  W.T @ x.T  matmuls)
  v      : natural [T on partitions, head-dims on free], with a ones-column
           appended per head so the PV matmul also produces the softmax
           denominator (lhsT = [v_h | 1] -> out rows 0..63 = y^T, row 64 = sum)
  S^T    : [keys on partitions, queries on free]; exp on ScalarE (no max
           subtraction needed: |S/8| <~ 6 for N(0,1) logits), causal mask via
           gpsimd affine_select on the diagonal tiles per query block.

Schedule (v2): single fused region.  Attention for query-block qb is
software-pipelined (S^T of step i+1 issued before PV of step i, so the PE
never waits on the ScalarE exp), with the qkv projection of t-block qb+1 and
the output projection of query-block qb-1 injected between attention steps as
filler matmuls to keep the PE saturated.  Exp is batched over both heads of a
head-pair ([128,1024] PSUM tile -> one ACTIVATE).  All PSUM evictions run on
DVE/ScalarE off the PE critical path.
"""

import numpy as np

import concourse.bass as bass
import concourse.mybir as mybir
import concourse.tile as tile
from concourse import bacc
from concourse.bass_utils import run_bass_kernel_spmd

B, T, C = 4, 2048, 1024
NH, HD = 16, 64
G = 2              # head groups (cores per batch)
HPG = NH // G      # heads per group = 8
GD = HPG * HD      # dims per group = 512
N_CORES = B * G

FP32 = mybir.dt.float32
BF16 = mybir.dt.bfloat16

NCC = C // 128      # 8 contraction chunks for the qkv projection
NMB = GD // 128     # 4 blocks of 128 qkv-dims per section (head-pairs)
NTB = T // 512      # 4 T-blocks of 512
NKC = T // 128      # 16 key chunks of 128


def build_nc():
    nc = bacc.Bacc()

    x = nc.declare_dram_parameter("x", [T, C], BF16, isOutput=False)
    wq = nc.declare_dram_parameter("wq", [C, GD], BF16, isOutput=False)
    wk = nc.declare_dram_parameter("wk", [C, GD], BF16, isOutput=False)
    wv = nc.declare_dram_parameter("wv", [C, GD], BF16, isOutput=False)
    bq = nc.declare_dram_parameter("bq", [GD], FP32, isOutput=False)
    bk = nc.declare_dram_parameter("bk", [GD], FP32, isOutput=False)
    bv = nc.declare_dram_parameter("bv", [GD], FP32, isOutput=False)
    wp = nc.declare_dram_parameter("wp", [GD, C], BF16, isOutput=False)
    out = nc.declare_dram_parameter("out", [T, C], FP32, isOutput=True)

    from contextlib import ExitStack

    with tile.TileContext(nc) as tc, ExitStack() as stack:
        consts = stack.enter_context(tc.tile_pool(name="consts", bufs=1))
        persist = stack.enter_context(tc.tile_pool(name="persist", bufs=1))
        wA_pool = stack.enter_context(tc.tile_pool(name="wA", bufs=1))
        xT_pool = stack.enter_context(tc.tile_pool(name="xT", bufs=16))
        pT_pool = stack.enter_context(tc.tile_pool(name="pT", bufs=4))
        nrm_pool = stack.enter_context(tc.tile_pool(name="nrm", bufs=4))
        osb_pool = stack.enter_context(tc.tile_pool(name="osb", bufs=4))
        accps_pool = stack.enter_context(
            tc.tile_pool(name="accps", bufs=2, space="PSUM")
        )
        sps_pool = stack.enter_context(tc.tile_pool(name="sps", bufs=2, space="PSUM"))
        pvps_pool = stack.enter_context(
            tc.tile_pool(name="pvps", bufs=2, space="PSUM")
        )

        # ---- weights (scalar queue: parallel with x transposes on sync) ----
        wq_t = wA_pool.tile([128, NCC, GD], BF16, tag="wq")
        wk_t = wA_pool.tile([128, NCC, GD], BF16, tag="wk")
        wv_t = wA_pool.tile([128, NCC, GD], BF16, tag="wv")
        wp_t = wA_pool.tile([128, NMB, C], BF16, tag="wp")
        nc.scalar.dma_start(out=wk_t, in_=wk[:, :].rearrange("(c p) d -> p c d", p=128))
        nc.scalar.dma_start(out=wq_t, in_=wq[:, :].rearrange("(c p) d -> p c d", p=128))
        nc.scalar.dma_start(out=wv_t, in_=wv[:, :].rearrange("(c p) d -> p c d", p=128))
        nc.scalar.dma_start(out=wp_t, in_=wp[:, :].rearrange("(c p) d -> p c d", p=128))

        # ---- biases (small; needed only at first eviction) ----
        bq_col = consts.tile([128, NMB], FP32, tag="bq_col")
        bk_col = consts.tile([128, NMB], FP32, tag="bk_col")
        for m in range(NMB):
            nc.scalar.dma_start(out=bq_col[:, m : m + 1], in_=bq[bass.ts(m, 128)])
            nc.scalar.dma_start(out=bk_col[:, m : m + 1], in_=bk[bass.ts(m, 128)])
        bv_bc = consts.tile([128, GD], FP32, tag="bv_bc")
        nc.scalar.dma_start(out=bv_bc, in_=bv[None, :].partition_broadcast(128))

        # ---- persistent activations ----
        qT_t = [persist.tile([128, T], BF16, tag=f"qT{m}", name=f"qT{m}") for m in range(NMB)]
        kT_t = [persist.tile([128, T], BF16, tag=f"kT{m}", name=f"kT{m}") for m in range(NMB)]
        yT_t = [persist.tile([128, T], BF16, tag=f"yT{m}", name=f"yT{m}") for m in range(NMB)]
        v_all = persist.tile([128, NKC, HPG, HD + 1], BF16, tag="v_all")
        # softmax-denominator ones column for every key chunk / head
        nc.gpsimd.memset(v_all[:, :, :, HD : HD + 1], 1.0)

        # ---------------- emission helpers ----------------

        def xtc_dma(tb):
            """Issue the x^T transpose DMAs for t-block tb; returns tiles."""
            xtc = [
                xT_pool.tile([128, 512], BF16, tag="xtc", name=f"xtc{tb}_{c}")
                for c in range(NCC)
            ]
            for c in range(NCC):
                nc.sync.dma_start(
                    out=xtc[c],
                    in_=x[bass.ts(tb, 512), bass.ts(c, 128)],
                    transpose=True,
                )
            return xtc

        def qkv_ops(tb, xtc):
            """One-PE-matmul callables for the qkv projection of t-block tb."""
            ops = []

            def qk_chain(w_t, b_col, dst, m):
                ps = accps_pool.tile([128, 512], FP32, tag="accps", name="accps")

                def mk(c):
                    def op():
                        nc.tensor.matmul(
                            ps,
                            w_t[:, c, bass.ts(m, 128)],
                            xtc[c],
                            start=(c == 0),
                            stop=(c == NCC - 1),
                        )
                        if c == NCC - 1:
                            nc.vector.tensor_scalar_add(
                                dst[m][:, bass.ts(tb, 512)], ps, b_col[:, m : m + 1]
                            )

                    return op

                return [mk(c) for c in range(NCC)]

            def v_chain(tsub):
                kc = tb * 4 + tsub
                ps = accps_pool.tile([128, GD], FP32, tag="accps", name="accps")

                def mk(c):
                    def op():
                        nc.tensor.matmul(
                            ps,
                            xtc[c][:, bass.ts(tsub, 128)],
                            wv_t[:, c, :],
                            start=(c == 0),
                            stop=(c == NCC - 1),
                        )
                        if c == NCC - 1:
                            vt = v_all[:, kc, :, :]
                            nc.vector.tensor_add(
                                vt[:, :, 0:HD],
                                ps.rearrange("p (h d) -> p h d", h=HPG),
                                bv_bc.rearrange("p (h d) -> p h d", h=HPG),
                            )

                    return op

                return [mk(c) for c in range(NCC)]

            for m in range(NMB):
                ops += qk_chain(wk_t, bk_col, kT_t, m)
                ops += qk_chain(wq_t, bq_col, qT_t, m)
                ops += v_chain(m)
            return ops

        def proj_ops(qb):
            """One-PE-matmul callables for the output projection of block qb."""
            ops = []
            for tsub in range(4):
                tb16 = qb * 4 + tsub
                for nb in range(C // 512):
                    ps = accps_pool.tile([128, 512], FP32, tag="accps", name="accps")

                    def mk(ps, tb16, nb, c):
                        def op():
                            nc.tensor.matmul(
                                ps,
                                yT_t[c][:, bass.ts(tb16, 128)],
                                wp_t[:, c, bass.ts(nb, 512)],
                                start=(c == 0),
                                stop=(c == NMB - 1),
                            )
                            if c == NMB - 1:
                                osb = osb_pool.tile([128, 512], FP32, tag="osb", name="osb")
                                nc.scalar.copy(osb, ps)
                                nc.sync.dma_start(
                                    out=out[bass.ts(tb16, 128), bass.ts(nb, 512)],
                                    in_=osb,
                                )

                        return op

                    ops += [mk(ps, tb16, nb, c) for c in range(NMB)]
            return ops

        # ---------------- attention ----------------
        scale = 1.0 / float(np.sqrt(HD))

        def attention(qb, fillers):
            kcmax = (qb + 1) * 4
            steps = [(m, kc) for m in range(NMB) for kc in range(kcmax)]
            nsteps = len(steps)
            pvs_by_m = {}
            pT_by = {}

            def emit_S(m, kc):
                sp = sps_pool.tile([128, 1024], FP32, tag="sps", name="sps")
                for hp in range(2):
                    base = hp * 64
                    nc.tensor.matmul(
                        sp[:, bass.ts(hp, 512)],
                        kT_t[m][base : base + 64, bass.ts(kc, 128)],
                        qT_t[m][base : base + 64, bass.ts(qb, 512)],
                        start=True,
                        stop=True,
                    )
                pT = pT_pool.tile([128, 1024], BF16, tag="pT", name="pT")
                nc.scalar.activation(
                    out=pT, in_=sp, func=mybir.ActivationFunctionType.Exp, scale=scale
                )
                r = kc - qb * 4
                if r >= 0:
                    # keep key j <= query i within the diagonal stripe; columns
                    # beyond 128*(r+1) are already fully valid.
                    w = 128 * (r + 1)
                    for hp in range(2):
                        sl = pT[:, 512 * hp : 512 * hp + w]
                        nc.gpsimd.affine_select(
                            out=sl,
                            in_=sl,
                            compare_op=mybir.AluOpType.is_ge,
                            fill=0.0,
                            base=-128 * r,
                            channel_multiplier=-1,
                            pattern=[[1, w]],
                        )
                pT_by[(m, kc)] = pT

            def emit_P(m, kc):
                if kc == 0:
                    pvs_by_m[m] = [
                        pvps_pool.tile([HD + 1, 512], FP32, tag="pvps", name="pvps")
                        for _ in range(2)
                    ]
                pvs = pvs_by_m[m]
                pT = pT_by.pop((m, kc))
                for hp in range(2):
                    nc.tensor.matmul(
                        pvs[hp],
                        v_all[:, kc, 2 * m + hp, :],
                        pT[:, bass.ts(hp, 512)],
                        start=(kc == 0),
                        stop=(kc == kcmax - 1),
                    )
                if kc == kcmax - 1:
                    # evict y^T (unnormalized) and normalize by the softmax
                    # denominator accumulated in row 64.
                    for hp in range(2):
                        base = hp * 64
                        ycols = yT_t[m][base : base + 64, bass.ts(qb, 512)]
                        nc.vector.tensor_copy(ycols, pvs[hp][0:HD, :])
                        den_s = nrm_pool.tile([1, 512], FP32, tag="den_s", name="den_s")
                        nc.vector.tensor_copy(den_s, pvs[hp][HD : HD + 1, :])
                        denr = nrm_pool.tile([1, 512], FP32, tag="denr", name="denr")
                        nc.vector.reciprocal_approx_fast(out=denr, in_=den_s)
                        rbc = nrm_pool.tile([128, 512], FP32, tag="rbc", name="rbc")
                        nc.gpsimd.partition_broadcast(rbc, denr)
                        nc.vector.tensor_mul(ycols, ycols, rbc[base : base + 64, :])

            nfill = len(fillers)
            fi = 0
            prev = None
            for i, st in enumerate(steps):
                emit_S(*st)
                # inject filler PE work between the S^T and the dependent PV
                want = (nfill * (i + 1)) // nsteps
                while fi < want:
                    fillers[fi]()
                    fi += 1
                if prev is not None:
                    emit_P(*prev)
                prev = st
            while fi < nfill:
                fillers[fi]()
                fi += 1
            emit_P(*prev)

        # ---------------- top-level schedule ----------------
        xtc_cur = xtc_dma(0)
        for op in qkv_ops(0, xtc_cur):
            op()
        for qb in range(NTB):
            fillers = []
            if qb < NTB - 1:
                xtc_nxt = xtc_dma(qb + 1)
                fillers += qkv_ops(qb + 1, xtc_nxt)
            if qb > 0:
                fillers += proj_ops(qb - 1)
            attention(qb, fillers)
        for op in proj_ops(NTB - 1):
            op()

    nc.compile()
    return nc


_CACHE = {}


def _get_nc():
    if "nc" not in _CACHE:
        _CACHE["nc"] = build_nc()
    return _CACHE["nc"]


def _to_bf16(a):
    import ml_dtypes

    a = np.asarray(a, dtype=np.float32)
    return np.ascontiguousarray(a.astype(ml_dtypes.bfloat16))


def make_in_maps(x, w_qkv, b_qkv, w_proj):
    x = np.asarray(x, dtype=np.float32)
    w_qkv = np.asarray(w_qkv, dtype=np.float32)
    b_qkv = np.asarray(b_qkv, dtype=np.float32)
    in_maps = []
    for core in range(N_CORES):
        b, g = divmod(core, G)
        in_maps.append(
            {
                "x": _to_bf16(x[b]),
                "wq": _to_bf16(w_qkv[:, GD * g : GD * g + GD]),
                "wk": _to_bf16(w_qkv[:, C + GD * g : C + GD * g + GD]),
                "wv": _to_bf16(w_qkv[:, 2 * C + GD * g : 2 * C + GD * g + GD]),
                "bq": np.ascontiguousarray(b_qkv[GD * g : GD * g + GD]),
                "bk": np.ascontiguousarray(b_qkv[C + GD * g : C + GD * g + GD]),
                "bv": np.ascontiguousarray(b_qkv[2 * C + GD * g : 2 * C + GD * g + GD]),
                "wp": _to_bf16(np.asarray(w_proj, dtype=np.float32)[GD * g : GD * g + GD, :]),
            }
        )
    return in_maps


def _assemble(results, b_proj):
    y = np.empty((B, T, C), dtype=np.float32)
    for b in range(B):
        y[b] = results[G * b]["out"] + results[G * b + 1]["out"]
    y += np.asarray(b_proj, dtype=np.float32)[None, None, :]
    return y


def kernel(x, w_qkv, b_qkv, w_proj, b_proj):
    nc = _get_nc()
    in_maps = make_in_maps(x, w_qkv, b_qkv, w_proj)
    res = run_bass_kernel_spmd(nc, in_maps, list(range(N_CORES)))
    return _assemble(res.results, b_proj)


# revision 11
# speedup vs baseline: 1.6575x; 1.1014x over previous
"""Causal self-attention (B=4, T=2048, C=1024, NH=16) on 8 TRN2 NeuronCores.

Sharding: core = 2*b + g  (b in 0..3 batches, g in 0..1 head-groups of 8 heads).
Each core computes qkv projection for its 8 heads, causal flash attention,
and a partial output projection (rows g*512:(g+1)*512 of w_proj).  Host sums
the two partials per batch and adds b_proj.

Layouts on device (per core):
  qT, kT : [head-dims on partitions, T on free]  (from  W.T @ x.T  matmuls)
  v      : natural [T on partitions, head-dims on free], with a ones-column
           appended per head so the PV matmul also produces the softmax
           denominator (lhsT = [v_h | 1] -> out rows 0..63 = y^T, row 64 = sum)
  S^T    : [keys on partitions, queries on free]; exp on ScalarE (no max
           subtraction needed: |S/8| <~ 6 for N(0,1) logits), causal mask via
           gpsimd affine_select on the diagonal tiles per query block.

Schedule (v2): single fused region.  Attention for query-block qb is
software-pipelined (S^T of step i+1 issued before PV of step i, so the PE
never waits on the ScalarE exp), with the qkv projection of t-block qb+1 and
the output projection of query-block qb-1 injected between attention steps as
filler matmuls to keep the PE saturated.  Exp is batched over both heads of a
head-pair ([128,1024] PSUM tile -> one ACTIVATE).  All PSUM evictions run on
DVE/ScalarE off the PE critical path.
"""

import numpy as np

import concourse.bass as bass
import concourse.mybir as mybir
import concourse.tile as tile
from concourse import bacc
from concourse.bass_utils import run_bass_kernel_spmd

B, T, C = 4, 2048, 1024
NH, HD = 16, 64
G = 2              # head groups (cores per batch)
HPG = NH // G      # heads per group = 8
GD = HPG * HD      # dims per group = 512
N_CORES = B * G

FP32 = mybir.dt.float32
BF16 = mybir.dt.bfloat16

NCC = C // 128      # 8 contraction chunks for the qkv projection
NMB = GD // 128     # 4 blocks of 128 qkv-dims per section (head-pairs)
NTB = T // 512      # 4 T-blocks of 512
NKC = T // 128      # 16 key chunks of 128


def build_nc():
    nc = bacc.Bacc()

    xT = nc.declare_dram_parameter("xT", [C, T], BF16, isOutput=False)
    wq = nc.declare_dram_parameter("wq", [C, GD], BF16, isOutput=False)
    wk = nc.declare_dram_parameter("wk", [C, GD], BF16, isOutput=False)
    wv = nc.declare_dram_parameter("wv", [C, GD], BF16, isOutput=False)
    bq = nc.declare_dram_parameter("bq", [GD], FP32, isOutput=False)
    bk = nc.declare_dram_parameter("bk", [GD], FP32, isOutput=False)
    bv = nc.declare_dram_parameter("bv", [GD], FP32, isOutput=False)
    wp = nc.declare_dram_parameter("wp", [GD, C], BF16, isOutput=False)
    out = nc.declare_dram_parameter("out", [T, C], FP32, isOutput=True)

    from contextlib import ExitStack

    with tile.TileContext(nc) as tc, ExitStack() as stack:
        consts = stack.enter_context(tc.tile_pool(name="consts", bufs=1))
        persist = stack.enter_context(tc.tile_pool(name="persist", bufs=1))
        wA_pool = stack.enter_context(tc.tile_pool(name="wA", bufs=1))
        xT_pool = stack.enter_context(tc.tile_pool(name="xT", bufs=2))
        pT_pool = stack.enter_context(tc.tile_pool(name="pT", bufs=4))
        nrm_pool = stack.enter_context(tc.tile_pool(name="nrm", bufs=4))
        osb_pool = stack.enter_context(tc.tile_pool(name="osb", bufs=4))
        accps_pool = stack.enter_context(
            tc.tile_pool(name="accps", bufs=2, space="PSUM")
        )
        sps_pool = stack.enter_context(tc.tile_pool(name="sps", bufs=2, space="PSUM"))
        pvps_pool = stack.enter_context(
            tc.tile_pool(name="pvps", bufs=2, space="PSUM")
        )

        # ---- weights (scalar queue: parallel with x transposes on sync) ----
        wq_t = wA_pool.tile([128, NCC, GD], BF16, tag="wq")
        wk_t = wA_pool.tile([128, NCC, GD], BF16, tag="wk")
        wv_t = wA_pool.tile([128, NCC, GD], BF16, tag="wv")
        wp_t = wA_pool.tile([128, NMB, C], BF16, tag="wp")
        nc.scalar.dma_start(out=wk_t, in_=wk[:, :].rearrange("(c p) d -> p c d", p=128))
        nc.scalar.dma_start(out=wq_t, in_=wq[:, :].rearrange("(c p) d -> p c d", p=128))
        nc.scalar.dma_start(out=wv_t, in_=wv[:, :].rearrange("(c p) d -> p c d", p=128))
        nc.scalar.dma_start(out=wp_t, in_=wp[:, :].rearrange("(c p) d -> p c d", p=128))

        # ---- biases (small; needed only at first eviction) ----
        bq_col = consts.tile([128, NMB], FP32, tag="bq_col")
        bk_col = consts.tile([128, NMB], FP32, tag="bk_col")
        for m in range(NMB):
            nc.scalar.dma_start(out=bq_col[:, m : m + 1], in_=bq[bass.ts(m, 128)])
            nc.scalar.dma_start(out=bk_col[:, m : m + 1], in_=bk[bass.ts(m, 128)])
        bv_bc = consts.tile([128, GD], FP32, tag="bv_bc")
        nc.scalar.dma_start(out=bv_bc, in_=bv[None, :].partition_broadcast(128))

        # ---- persistent activations ----
        qT_t = [persist.tile([128, T], BF16, tag=f"qT{m}", name=f"qT{m}") for m in range(NMB)]
        kT_t = [persist.tile([128, T], BF16, tag=f"kT{m}", name=f"kT{m}") for m in range(NMB)]
        yT_t = [persist.tile([128, T], BF16, tag=f"yT{m}", name=f"yT{m}") for m in range(NMB)]
        v_all = persist.tile([128, NKC, HPG, HD + 1], BF16, tag="v_all")
        # softmax-denominator ones column for every key chunk / head
        nc.gpsimd.memset(v_all[:, :, :, HD : HD + 1], 1.0)

        # ---------------- emission helpers ----------------

        def xtc_dma(tb):
            """Load the x^T chunks for t-block tb in one DMA; returns tile."""
            xtc = xT_pool.tile([128, NCC, 512], BF16, tag="xtc", name=f"xtc{tb}")
            nc.sync.dma_start(
                out=xtc,
                in_=xT[:, bass.ts(tb, 512)].rearrange("(c p) t -> p c t", p=128),
            )
            return xtc

        def qkv_ops(tb, xtc):
            """One-PE-matmul callables for the qkv projection of t-block tb."""
            ops = []

            def qk_chain(w_t, b_col, dst, m):
                ps = accps_pool.tile([128, 512], FP32, tag="accps", name="accps")

                def mk(c):
                    def op():
                        nc.tensor.matmul(
                            ps,
                            w_t[:, c, bass.ts(m, 128)],
                            xtc[:, c, :],
                            start=(c == 0),
                            stop=(c == NCC - 1),
                        )
                        if c == NCC - 1:
                            nc.vector.tensor_scalar_add(
                                dst[m][:, bass.ts(tb, 512)], ps, b_col[:, m : m + 1]
                            )

                    return op

                return [mk(c) for c in range(NCC)]

            def v_chain(tsub):
                kc = tb * 4 + tsub
                ps = accps_pool.tile([128, GD], FP32, tag="accps", name="accps")

                def mk(c):
                    def op():
                        nc.tensor.matmul(
                            ps,
                            xtc[:, c, bass.ts(tsub, 128)],
                            wv_t[:, c, :],
                            start=(c == 0),
                            stop=(c == NCC - 1),
                        )
                        if c == NCC - 1:
                            vt = v_all[:, kc, :, :]
                            nc.vector.tensor_add(
                                vt[:, :, 0:HD],
                                ps.rearrange("p (h d) -> p h d", h=HPG),
                                bv_bc.rearrange("p (h d) -> p h d", h=HPG),
                            )

                    return op

                return [mk(c) for c in range(NCC)]

            for m in range(NMB):
                ops += qk_chain(wk_t, bk_col, kT_t, m)
                ops += qk_chain(wq_t, bq_col, qT_t, m)
                ops += v_chain(m)
            return ops

        def proj_ops(qb):
            """One-PE-matmul callables for the output projection of block qb."""
            ops = []
            for tsub in range(4):
                tb16 = qb * 4 + tsub
                for nb in range(C // 512):
                    ps = accps_pool.tile([128, 512], FP32, tag="accps", name="accps")

                    def mk(ps, tb16, nb, c):
                        def op():
                            nc.tensor.matmul(
                                ps,
                                yT_t[c][:, bass.ts(tb16, 128)],
                                wp_t[:, c, bass.ts(nb, 512)],
                                start=(c == 0),
                                stop=(c == NMB - 1),
                            )
                            if c == NMB - 1:
                                osb = osb_pool.tile([128, 512], FP32, tag="osb", name="osb")
                                nc.scalar.copy(osb, ps)
                                nc.sync.dma_start(
                                    out=out[bass.ts(tb16, 128), bass.ts(nb, 512)],
                                    in_=osb,
                                )

                        return op

                    ops += [mk(ps, tb16, nb, c) for c in range(NMB)]
            return ops

        # ---------------- attention ----------------
        scale = 1.0 / float(np.sqrt(HD))

        def attention(qb, fillers):
            kcmax = (qb + 1) * 4
            steps = [(m, kc) for m in range(NMB) for kc in range(kcmax)]
            nsteps = len(steps)
            pvs_by_m = {}
            pT_by = {}

            def emit_S(m, kc):
                sp = sps_pool.tile([128, 1024], FP32, tag="sps", name="sps")
                for hp in range(2):
                    base = hp * 64
                    nc.tensor.matmul(
                        sp[:, bass.ts(hp, 512)],
                        kT_t[m][base : base + 64, bass.ts(kc, 128)],
                        qT_t[m][base : base + 64, bass.ts(qb, 512)],
                        start=True,
                        stop=True,
                    )
                pT = pT_pool.tile([128, 1024], BF16, tag="pT", name="pT")
                nc.scalar.activation(
                    out=pT, in_=sp, func=mybir.ActivationFunctionType.Exp, scale=scale
                )
                r = kc - qb * 4
                if r >= 0:
                    # keep key j <= query i within the diagonal stripe; columns
                    # beyond 128*(r+1) are already fully valid.
                    w = 128 * (r + 1)
                    for hp in range(2):
                        sl = pT[:, 512 * hp : 512 * hp + w]
                        nc.gpsimd.affine_select(
                            out=sl,
                            in_=sl,
                            compare_op=mybir.AluOpType.is_ge,
                            fill=0.0,
                            base=-128 * r,
                            channel_multiplier=-1,
                            pattern=[[1, w]],
                        )
                pT_by[(m, kc)] = pT

            def emit_P(m, kc):
                if kc == 0:
                    pvs_by_m[m] = [
                        pvps_pool.tile([HD + 1, 512], FP32, tag="pvps", name="pvps")
                        for _ in range(2)
                    ]
                pvs = pvs_by_m[m]
                pT = pT_by.pop((m, kc))
                for hp in range(2):
                    nc.tensor.matmul(
                        pvs[hp],
                        v_all[:, kc, 2 * m + hp, :],
                        pT[:, bass.ts(hp, 512)],
                        start=(kc == 0),
                        stop=(kc == kcmax - 1),
                    )
                if kc == kcmax - 1:
                    # evict y^T (unnormalized) and normalize by the softmax
                    # denominator accumulated in row 64.
                    for hp in range(2):
                        base = hp * 64
                        ycols = yT_t[m][base : base + 64, bass.ts(qb, 512)]
                        nc.vector.tensor_copy(ycols, pvs[hp][0:HD, :])
                        den_s = nrm_pool.tile([1, 512], FP32, tag="den_s", name="den_s")
                        nc.vector.tensor_copy(den_s, pvs[hp][HD : HD + 1, :])
                        denr = nrm_pool.tile([1, 512], FP32, tag="denr", name="denr")
                        nc.vector.reciprocal_approx_fast(out=denr, in_=den_s)
                        rbc = nrm_pool.tile([128, 512], FP32, tag="rbc", name="rbc")
                        nc.gpsimd.partition_broadcast(rbc, denr)
                        nc.vector.tensor_mul(ycols, ycols, rbc[base : base + 64, :])

            nfill = len(fillers)
            fi = 0
            prev = None
            for i, st in enumerate(steps):
                emit_S(*st)
                # inject filler PE work between the S^T and the dependent PV
                want = (nfill * (i + 1)) // nsteps
                while fi < want:
                    fillers[fi]()
                    fi += 1
                if prev is not None:
                    emit_P(*prev)
                prev = st
            while fi < nfill:
                fillers[fi]()
                fi += 1
            emit_P(*prev)

        # ---------------- top-level schedule ----------------
        xtc_cur = xtc_dma(0)
        for op in qkv_ops(0, xtc_cur):
            op()
        for qb in range(NTB):
            fillers = []
            if qb < NTB - 1:
                xtc_nxt = xtc_dma(qb + 1)
                fillers += qkv_ops(qb + 1, xtc_nxt)
            if qb > 0:
                fillers += proj_ops(qb - 1)
            attention(qb, fillers)
        for op in proj_ops(NTB - 1):
            op()

    nc.compile()
    return nc


_CACHE = {}


def _get_nc():
    if "nc" not in _CACHE:
        _CACHE["nc"] = build_nc()
    return _CACHE["nc"]


def _to_bf16(a):
    import ml_dtypes

    a = np.asarray(a, dtype=np.float32)
    return np.ascontiguousarray(a.astype(ml_dtypes.bfloat16))


def make_in_maps(x, w_qkv, b_qkv, w_proj):
    x = np.asarray(x, dtype=np.float32)
    w_qkv = np.asarray(w_qkv, dtype=np.float32)
    b_qkv = np.asarray(b_qkv, dtype=np.float32)
    in_maps = []
    for core in range(N_CORES):
        b, g = divmod(core, G)
        in_maps.append(
            {
                "xT": _to_bf16(x[b].T),
                "wq": _to_bf16(w_qkv[:, GD * g : GD * g + GD]),
                "wk": _to_bf16(w_qkv[:, C + GD * g : C + GD * g + GD]),
                "wv": _to_bf16(w_qkv[:, 2 * C + GD * g : 2 * C + GD * g + GD]),
                "bq": np.ascontiguousarray(b_qkv[GD * g : GD * g + GD]),
                "bk": np.ascontiguousarray(b_qkv[C + GD * g : C + GD * g + GD]),
                "bv": np.ascontiguousarray(b_qkv[2 * C + GD * g : 2 * C + GD * g + GD]),
                "wp": _to_bf16(np.asarray(w_proj, dtype=np.float32)[GD * g : GD * g + GD, :]),
            }
        )
    return in_maps


def _assemble(results, b_proj):
    y = np.empty((B, T, C), dtype=np.float32)
    for b in range(B):
        y[b] = results[G * b]["out"] + results[G * b + 1]["out"]
    y += np.asarray(b_proj, dtype=np.float32)[None, None, :]
    return y


def kernel(x, w_qkv, b_qkv, w_proj, b_proj):
    nc = _get_nc()
    in_maps = make_in_maps(x, w_qkv, b_qkv, w_proj)
    res = run_bass_kernel_spmd(nc, in_maps, list(range(N_CORES)))
    return _assemble(res.results, b_proj)
